# revision 8
# baseline (speedup 1.0000x reference)
"""Trainium2 Bass kernel for nn_Compression, v3 (fp8 phase A).

Computes: out = X + GAMMA * (P @ (P.T @ X)),  P = softmax(X @ W.T + b)

Strategy (8 NeuronCores, data-parallel over N):
  Phase A (per row-tile of 128 rows, software-pipelined):
    - DMA X tile (f32, kept resident for the residual).
    - DVE casts the tile to fp8e4 (the whole correction term is scaled
      by GAMMA=1e-4, so fp8's ~6% relative error contributes ~1e-5 to
      the output: far inside the 2e-2 gate).
    - PE-transposes the fp8 tile (fp8 identity), Pool drains PSUM->SBUF.
    - Logits via 4 fp8 DoubleRow matmuls (K=256 each) + bf16 bias matmul.
    - Softmax: ACT exp with row-sum accumulator, DVE reciprocal + scale,
      casting P directly to fp8.
    - P.T @ X accumulated into 4 resident PSUM banks via fp8 DoubleRow
      matmuls over row-tile PAIRS (K=256 = two row tiles per matmul).
    - P.T (for phase B) via PE transposes, drained to bf16 by ACT.
  - PtX partials drain to bf16 and AllReduce in TWO D-halves (bf16,
    256 KiB each) so phase B on half 0 overlaps the half-1 collective.
  Phase B (per row-tile, per D-half):
    - corr = P @ (gamma * PtX) in bf16 (lhsT = resident P.T).
    - ACT drains PSUM->SBUF, DVE adds the exact f32 residual, DMA out.

The host side only reshapes: shards X rows, passes W transposed (pure
relayout, still f32) and b as-is.
"""

import sys

import numpy as np

if "/opt/trn_rl_repo" not in sys.path:
    sys.path.insert(0, "/opt/trn_rl_repo")

N, D, C = 32768, 1024, 256
GAMMA = 1e-4
NCORES = 8
NLOC = N // NCORES  # 4096
P = 128
NT = NLOC // P  # 32
NPAIR = NT // 2  # 16
DH = 512

_cache = {}


def _build_nc():
    import concourse.tile as tile
    from concourse import bacc
    import concourse.mybir as mybir
    from concourse.masks import make_identity
    from contextlib import ExitStack

    f32 = mybir.dt.float32
    bf16 = mybir.dt.bfloat16
    fp8 = mybir.dt.float8e4
    DR = mybir.MatmulPerfMode.DoubleRow
    AF = mybir.ActivationFunctionType

    nc = bacc.Bacc("TRN2", target_bir_lowering=False, debug=False, num_devices=NCORES)
    X = nc.dram_tensor("X", [NLOC, D], f32, kind="ExternalInput").ap()
    Wt = nc.dram_tensor("Wt", [D, C], f32, kind="ExternalInput").ap()
    bvec = nc.dram_tensor("b", [C], f32, kind="ExternalInput").ap()
    out = nc.dram_tensor("out", [NLOC, D], f32, kind="ExternalOutput").ap()

    with tile.TileContext(nc) as tc, ExitStack() as ctx:
        const = ctx.enter_context(tc.tile_pool(name="const", bufs=1))
        xres = ctx.enter_context(tc.tile_pool(name="xres", bufs=1))
        # xqp holds an fp8 row-tile PAIR; written at load(2p)/load(2p+1),
        # read by transpose and by ptx(pair) ~3 steps later.
        xqp = ctx.enter_context(tc.tile_pool(name="xqp", bufs=3))
        xtp = ctx.enter_context(tc.tile_pool(name="xtp", bufs=2))
        ppool = ctx.enter_context(tc.tile_pool(name="ppool", bufs=4))
        pqp = ctx.enter_context(tc.tile_pool(name="pqp", bufs=3))
        spool = ctx.enter_context(tc.tile_pool(name="spool", bufs=4))
        opool = ctx.enter_context(tc.tile_pool(name="opool", bufs=6))
        dram = ctx.enter_context(tc.tile_pool(name="dram", bufs=1, space="DRAM"))

        ident = const.tile([P, P], fp8)
        make_identity(nc, ident)

        # W.T in fp8, [d-within-chunk, k-chunk, c]; d = k*128 + p. Loaded
        # in 4 parallel DMA chunks, cast on ScalarE.
        Wq = const.tile([P, 8, C], fp8)
        with tc.tile_pool(name="wtmp", bufs=1) as wtmp:
            wt_f = wtmp.tile([P, 8, C], f32)
            wt_r = Wt.rearrange("(k p) c -> p k c", p=P)
            for q in range(4):
                nc.sync.dma_start(wt_f[:, 2 * q:2 * q + 2, :], wt_r[:, 2 * q:2 * q + 2, :])
                nc.scalar.copy(Wq[:, 2 * q:2 * q + 2, :], wt_f[:, 2 * q:2 * q + 2, :])

        ones1 = const.tile([1, P], bf16)
        nc.vector.memset(ones1[:], 1.0)
        b_sb = const.tile([1, C], bf16)
        with tc.tile_pool(name="btmp", bufs=1) as btmp:
            b_f = btmp.tile([1, C], f32)
            nc.sync.dma_start(b_f[:], bvec.rearrange("(o c) -> o c", o=1))
            nc.vector.tensor_copy(b_sb[:], b_f[:])

        Xall = xres.tile([P, NT, D], f32)
        Pt = const.tile([P, 2, NLOC], bf16)  # P.T resident (c-chunk major)

        ar_in = [dram.tile([C, DH], bf16, name=f"ar_in{h}") for h in range(2)]
        ar_out = [
            dram.tile([C, DH], bf16, addr_space="Shared", name=f"ar_out{h}")
            for h in range(2)
        ]

        # Tiny warm-up AllReduce emitted first: absorbs the collectives
        # stream's one-time BARRIER/init (~50us) concurrently with phase A
        # so the real AllReduces start promptly at phase A's end.
        warm_in = dram.tile([1, 64], f32, name="warm_in")
        warm_out = dram.tile([1, 64], f32, addr_space="Shared", name="warm_out")
        with tc.tile_pool(name="wrm", bufs=1) as wrm:
            w_sb = wrm.tile([1, 64], f32)
            nc.vector.memset(w_sb[:], 0.0)
            nc.sync.dma_start(warm_in[:], w_sb[:])
        nc.gpsimd.collective_compute(
            "AllReduce",
            mybir.AluOpType.add,
            replica_groups=[list(range(NCORES))],
            ins=[warm_in[:].opt()],
            outs=[warm_out[:].opt()],
        )

        # ---- phase A ----
        def s_load(i):
            xi = Xall[:, i, :]
            nc.sync.dma_start(xi, X[i * P:(i + 1) * P, :])
            if i % 2 == 0:
                xq = xqp.tile([P, 2, D], fp8, name="xq", tag="xq")
            else:
                xq = None  # odd tile writes into the pair slot
            return xq

        def s_cast(i, xq):
            nc.vector.tensor_copy(xq[:, i % 2, :], Xall[:, i, :])

        def s_transpose(i, xq):
            # 8 PE transposes into one PSUM accumulation group; the
            # PSUM->SBUF drain alternates ACT/DVE (GpSimd has no PSUM port).
            # (fp8 transpose mode requires output element step of 2, so the
            # PSUM tile carries a trailing stride-2 dim; drains re-pack.)
            xt = xtp.tile([P, 8, P], fp8, name="xt", tag="xt")
            trp = psA.tile([P, 8, P, 2], fp8, name="trp", tag="trp")
            for k in range(8):
                nc.tensor.matmul(
                    trp[:, k, :, 0],
                    xq[:, i % 2, k * P:(k + 1) * P],
                    ident[:],
                    is_transpose=True,
                    start=(k == 0),
                    stop=(k == 7),
                )
            if i % 2 == 0:
                nc.scalar.copy(xt[:], trp[:, :, :, 0])
            else:
                nc.vector.tensor_copy(xt[:], trp[:, :, :, 0])
            return xt

        def s_logits(i, xt):
            lg = psL.tile([P, C], f32, name="lg", tag="lg")
            for q in range(4):
                nc.tensor.matmul(
                    lg[:],
                    xt[:, 2 * q:2 * q + 2, :],
                    Wq[:, 2 * q:2 * q + 2, :],
                    start=(q == 0),
                    stop=False,
                    perf_mode=DR,
                )
            nc.tensor.matmul(lg[:], ones1[:], b_sb[:], start=False, stop=True)
            return lg

        def s_softmax(i, lg, pq):
            # |logits| <= ~10 so exp is safe without max-subtraction
            p_sb = ppool.tile([P, C], f32, name="p_sb", tag="p")
            ssum = spool.tile([P, 1], f32, name="ssum", tag="s")
            nc.scalar.activation(p_sb[:], lg[:], AF.Exp, accum_out=ssum[:])
            rinv = spool.tile([P, 1], f32, name="rinv", tag="r")
            nc.vector.reciprocal(rinv[:], ssum[:])
            # normalization+fp8 cast on GpSimd (SBUF-only op) to keep
            # DVE/ACT free for the casts and drains
            nc.gpsimd.tensor_scalar_mul(pq[:, i % 2, :], p_sb[:], rinv[:])

        def s_ptx(pair, pq, xq):
            # PtX += P_pair.T @ X_pair, fp8 DoubleRow over K=256 rows
            for cc in range(2):
                for h in range(2):
                    nc.tensor.matmul(
                        ptx_ps[2 * cc + h][:],
                        pq[:, :, cc * P:(cc + 1) * P],
                        xq[:, :, h * DH:(h + 1) * DH],
                        start=(pair == 0),
                        stop=(pair == NPAIR - 1),
                        perf_mode=DR,
                    )
            # P.T for phase B: 4 transposes (2 tiles x 2 c-chunks), ACT
            # drains fp8 PSUM -> resident bf16 Pt.
            ptp = psP.tile([P, 2, C, 2], fp8, name="ptp", tag="ptp")
            for j in range(2):
                for cc in range(2):
                    nc.tensor.matmul(
                        ptp[:, j, cc * P:(cc + 1) * P, 0],
                        pq[:, j, cc * P:(cc + 1) * P],
                        ident[:],
                        is_transpose=True,
                        start=(j == 0 and cc == 0),
                        stop=(j == 1 and cc == 1),
                    )
            dst = Pt[:, :, 2 * pair * P:(2 * pair + 2) * P].rearrange(
                "p cc (j r) -> p cc j r", j=2
            )
            src = ptp[:, :, :, 0].rearrange("p j (cc r) -> p cc j r", cc=2)
            nc.scalar.copy(dst, src)

        with tc.tile_pool(name="psA", bufs=2, space="PSUM") as psA, \
             tc.tile_pool(name="psP", bufs=1, space="PSUM") as psP, \
             tc.tile_pool(name="psL", bufs=1, space="PSUM") as psL, \
             tc.tile_pool(name="psX", bufs=1, space="PSUM") as psX:
            ptx_ps = [
                psX.tile([P, DH], f32, name=f"ptx_{c}_{h}", tag=f"ptx_{c}_{h}")
                for c in range(2)
                for h in range(2)
            ]
            # Pipeline: at step i run logits(i), transpose(i+1), load(i+2),
            # and ptx for the pair ending at tile i-2. The 2-step skew lets
            # the ACT exp latency hide under PE work.
            state = {}

            def stage_front(i):
                # load + cast for tile i
                if i % 2 == 0:
                    xq = s_load(i)
                else:
                    s_load(i)
                    xq = state[i - 1][0]
                s_cast(i, xq)
                return xq

            xq0 = stage_front(0)
            state[0] = (xq0, s_transpose(0, xq0), None)
            xq1 = stage_front(1)
            state[1] = (xq1, None, None)
            for i in range(NT):
                xq_i, xt_i, _ = state[i]
                lg = s_logits(i, xt_i)
                if i % 2 == 0:
                    pq = pqp.tile([P, 2, C], fp8, name="pq", tag="pq")
                else:
                    pq = state[i - 1][2]
                s_softmax(i, lg, pq)
                state[i] = (xq_i, xt_i, pq)
                if i + 1 < NT:
                    xq_n, _, _ = state[i + 1]
                    state[i + 1] = (xq_n, s_transpose(i + 1, xq_n), None)
                if i + 2 < NT:
                    state[i + 2] = (stage_front(i + 2), None, None)
                if i >= 3 and (i - 2) % 2 == 1:
                    pair = (i - 3) // 2
                    xq_p = state.pop(2 * pair)[0]
                    pq_p = state.pop(2 * pair + 1)[2]
                    s_ptx(pair, pq_p, xq_p)
            # in-loop ptx covers pairs 0..NPAIR-2; finish the last pair
            for pair in (NPAIR - 1,):
                xq_p = state.pop(2 * pair)[0]
                pq_p = state.pop(2 * pair + 1)[2]
                s_ptx(pair, pq_p, xq_p)

            # PSUM -> SBUF -> DRAM bounce, one per D-half, cast to bf16
            # (the PtX rounding is gamma-scaled: invisible in the output).
            for h in range(2):
                s = const.tile([P, 2, DH], bf16, name=f"stg{h}", tag=f"stg{h}")
                nc.vector.tensor_copy(s[:, 0, :], ptx_ps[h][:])
                nc.scalar.copy(s[:, 1, :], ptx_ps[2 + h][:])
                nc.sync.dma_start(
                    ar_in[h].rearrange("(c p) d -> p c d", p=P), s[:]
                )

        # ---- phase B, interleaved with the collectives: AllReduce h=1 is
        # emitted AFTER phase B h=0 so h=0's consumers only wait on the
        # first collective's completion tick, and the second collective
        # runs concurrently with h=0 compute. gamma folded into PtX so the
        # residual is one add. ----
        def ar(h):
            nc.gpsimd.collective_compute(
                "AllReduce",
                mybir.AluOpType.add,
                replica_groups=[list(range(NCORES))],
                ins=[ar_in[h][:].opt()],
                outs=[ar_out[h][:].opt()],
            )

        def phase_b(h, psB, cpool):
            pall = const.tile([P, 2, DH], bf16, name=f"pall{h}", tag=f"stg{h}")
            nc.sync.dma_start(
                pall[:], ar_out[h].rearrange("(c p) d -> p c d", p=P)
            )
            ptxb = const.tile([P, 2, DH], bf16, name=f"ptxb{h}")
            nc.vector.tensor_scalar_mul(ptxb[:], pall[:], GAMMA)
            for i in range(NT):
                cor = psB.tile([P, DH], f32, name="cor", tag="cor")
                for c in range(2):
                    nc.tensor.matmul(
                        cor[:],
                        Pt[:, c, i * P:(i + 1) * P],
                        ptxb[:, c, :],
                        start=(c == 0),
                        stop=(c == 1),
                    )
                o_sb = opool.tile([P, DH], f32, name="o_sb", tag="o")
                if i % 2 == 0:
                    # ACT drains PSUM, DVE adds SBUF+SBUF
                    cors = cpool.tile([P, DH], f32, name="cors", tag="cs")
                    nc.scalar.copy(cors[:], cor[:])
                    nc.vector.tensor_add(
                        o_sb[:], cors[:], Xall[:, i, h * DH:(h + 1) * DH]
                    )
                else:
                    # DVE adds straight from PSUM (one fewer hop)
                    nc.vector.tensor_add(
                        o_sb[:], cor[:], Xall[:, i, h * DH:(h + 1) * DH]
                    )
                nc.sync.dma_start(out[i * P:(i + 1) * P, h * DH:(h + 1) * DH], o_sb[:])

        with tc.tile_pool(name="psB", bufs=8, space="PSUM") as psB, \
             tc.tile_pool(name="cpool", bufs=4) as cpool:
            ar(0)
            phase_b(0, psB, cpool)
            ar(1)
            phase_b(1, psB, cpool)

    nc.finalize()
    return nc


def _run(inputs, trace=False, **kwargs):
    from concourse import bass_utils

    if "nc" not in _cache:
        _cache["nc"] = _build_nc()
    nc = _cache["nc"]

    X = np.ascontiguousarray(np.asarray(inputs["X"], dtype=np.float32))
    W = np.ascontiguousarray(np.asarray(inputs["W"], dtype=np.float32))
    b = np.ascontiguousarray(np.asarray(inputs["b"], dtype=np.float32))
    Wt = np.ascontiguousarray(W.T)

    in_maps = [
        {"X": X[i * NLOC:(i + 1) * NLOC], "Wt": Wt, "b": b} for i in range(NCORES)
    ]
    res = bass_utils.run_bass_kernel_spmd(
        nc, in_maps, core_ids=list(range(NCORES)), trace=trace, **kwargs
    )
    outp = np.concatenate([res.results[i]["out"] for i in range(NCORES)], axis=0)
    return outp, res


def kernel(**inputs):
    outp, _ = _run(inputs, trace=False)
    return outp


# revision 9
# speedup vs baseline: 1.4925x; 1.4925x over previous
"""Trainium2 Bass kernel for nn_Compression, v3 (fp8 phase A).

Computes: out = X + GAMMA * (P @ (P.T @ X)),  P = softmax(X @ W.T + b)

Strategy (8 NeuronCores, data-parallel over N):
  Phase A (per row-tile of 128 rows, software-pipelined):
    - DMA X tile (f32, kept resident for the residual).
    - DVE casts the tile to fp8e4 (the whole correction term is scaled
      by GAMMA=1e-4, so fp8's ~6% relative error contributes ~1e-5 to
      the output: far inside the 2e-2 gate).
    - PE-transposes the fp8 tile (fp8 identity), Pool drains PSUM->SBUF.
    - Logits via 4 fp8 DoubleRow matmuls (K=256 each) + bf16 bias matmul.
    - Softmax: ACT exp with row-sum accumulator, DVE reciprocal + scale,
      casting P directly to fp8.
    - P.T @ X accumulated into 4 resident PSUM banks via fp8 DoubleRow
      matmuls over row-tile PAIRS (K=256 = two row tiles per matmul).
    - P.T (for phase B) via PE transposes, drained to bf16 by ACT.
  - PtX partials drain to bf16 and AllReduce in TWO D-halves (bf16,
    256 KiB each) so phase B on half 0 overlaps the half-1 collective.
  Phase B (per row-tile, per D-half):
    - corr = P @ (gamma * PtX) in bf16 (lhsT = resident P.T).
    - ACT drains PSUM->SBUF, DVE adds the exact f32 residual, DMA out.

The host side only reshapes: shards X rows, passes W transposed (pure
relayout, still f32) and b as-is.
"""

import sys

import numpy as np

if "/opt/trn_rl_repo" not in sys.path:
    sys.path.insert(0, "/opt/trn_rl_repo")

N, D, C = 32768, 1024, 256
GAMMA = 1e-4
NCORES = 8
NLOC = N // NCORES  # 4096
P = 128
NT = NLOC // P  # 32
NPAIR = NT // 2  # 16
DH = 512

_cache = {}


def _build_nc():
    import concourse.tile as tile
    from concourse import bacc
    import concourse.mybir as mybir
    from concourse.masks import make_identity
    from contextlib import ExitStack

    f32 = mybir.dt.float32
    bf16 = mybir.dt.bfloat16
    fp8 = mybir.dt.float8e4
    DR = mybir.MatmulPerfMode.DoubleRow
    AF = mybir.ActivationFunctionType

    nc = bacc.Bacc("TRN2", target_bir_lowering=False, debug=False, num_devices=NCORES)
    X = nc.dram_tensor("X", [NLOC, D], f32, kind="ExternalInput").ap()
    Wt = nc.dram_tensor("Wt", [D, C], f32, kind="ExternalInput").ap()
    bvec = nc.dram_tensor("b", [C], f32, kind="ExternalInput").ap()
    out = nc.dram_tensor("out", [NLOC, D], f32, kind="ExternalOutput").ap()

    with tile.TileContext(nc) as tc, ExitStack() as ctx:
        const = ctx.enter_context(tc.tile_pool(name="const", bufs=1))
        xres = ctx.enter_context(tc.tile_pool(name="xres", bufs=1))
        # xqp holds an fp8 row-tile PAIR; written at load(2p)/load(2p+1),
        # read by transpose and by ptx(pair) ~3 steps later.
        xqp = ctx.enter_context(tc.tile_pool(name="xqp", bufs=3))
        xtp = ctx.enter_context(tc.tile_pool(name="xtp", bufs=2))
        ppool = ctx.enter_context(tc.tile_pool(name="ppool", bufs=4))
        pqp = ctx.enter_context(tc.tile_pool(name="pqp", bufs=3))
        spool = ctx.enter_context(tc.tile_pool(name="spool", bufs=4))
        opool = ctx.enter_context(tc.tile_pool(name="opool", bufs=6))
        dram = ctx.enter_context(tc.tile_pool(name="dram", bufs=1, space="DRAM"))

        ident = const.tile([P, P], fp8)
        make_identity(nc, ident)

        # W.T in fp8, [d-within-chunk, k-chunk, c]; d = k*128 + p. Loaded
        # in 4 parallel DMA chunks, cast on ScalarE.
        Wq = const.tile([P, 8, C], fp8)
        with tc.tile_pool(name="wtmp", bufs=1) as wtmp:
            wt_f = wtmp.tile([P, 8, C], f32)
            wt_r = Wt.rearrange("(k p) c -> p k c", p=P)
            for q in range(4):
                nc.sync.dma_start(wt_f[:, 2 * q:2 * q + 2, :], wt_r[:, 2 * q:2 * q + 2, :])
                nc.scalar.copy(Wq[:, 2 * q:2 * q + 2, :], wt_f[:, 2 * q:2 * q + 2, :])

        ones1 = const.tile([1, P], bf16)
        nc.vector.memset(ones1[:], 1.0)
        b_sb = const.tile([1, C], bf16)
        with tc.tile_pool(name="btmp", bufs=1) as btmp:
            b_f = btmp.tile([1, C], f32)
            nc.sync.dma_start(b_f[:], bvec.rearrange("(o c) -> o c", o=1))
            nc.vector.tensor_copy(b_sb[:], b_f[:])

        Xall = xres.tile([P, NT, D], f32)
        Pt = const.tile([P, 2, NLOC], bf16)  # P.T resident (c-chunk major)

        ar_in = [dram.tile([C, DH], bf16, name=f"ar_in{h}") for h in range(2)]
        ar_out = [
            dram.tile([C, DH], bf16, addr_space="Shared", name=f"ar_out{h}")
            for h in range(2)
        ]

        # Tiny warm-up AllReduce emitted first: absorbs the collectives
        # stream's one-time BARRIER/init (~50us) concurrently with phase A
        # so the real AllReduces start promptly at phase A's end.
        warm_in = dram.tile([1, 64], f32, name="warm_in")
        warm_out = dram.tile([1, 64], f32, addr_space="Shared", name="warm_out")
        with tc.tile_pool(name="wrm", bufs=1) as wrm:
            w_sb = wrm.tile([1, 64], f32)
            nc.vector.memset(w_sb[:], 0.0)
            nc.sync.dma_start(warm_in[:], w_sb[:])
        nc.gpsimd.collective_compute(
            "AllReduce",
            mybir.AluOpType.add,
            replica_groups=[list(range(NCORES))],
            ins=[warm_in[:].opt()],
            outs=[warm_out[:].opt()],
        )

        # ---- phase A ----
        def s_load(i):
            xi = Xall[:, i, :]
            nc.sync.dma_start(xi, X[i * P:(i + 1) * P, :])
            if i % 2 == 0:
                xq = xqp.tile([P, 2, D], fp8, name="xq", tag="xq")
            else:
                xq = None  # odd tile writes into the pair slot
            return xq

        def s_cast(i, xq):
            nc.vector.tensor_copy(xq[:, i % 2, :], Xall[:, i, :])

        def s_transpose(i, xq):
            # 8 PE transposes into one PSUM accumulation group; the
            # PSUM->SBUF drain alternates ACT/DVE (GpSimd has no PSUM port).
            # (fp8 transpose mode requires output element step of 2, so the
            # PSUM tile carries a trailing stride-2 dim; drains re-pack.)
            xt = xtp.tile([P, 8, P], fp8, name="xt", tag="xt")
            trp = psA.tile([P, 8, P, 2], fp8, name="trp", tag="trp")
            for k in range(8):
                nc.tensor.matmul(
                    trp[:, k, :, 0],
                    xq[:, i % 2, k * P:(k + 1) * P],
                    ident[:],
                    is_transpose=True,
                    start=(k == 0),
                    stop=(k == 7),
                )
            if i % 2 == 0:
                nc.scalar.copy(xt[:], trp[:, :, :, 0])
            else:
                nc.vector.tensor_copy(xt[:], trp[:, :, :, 0])
            return xt

        def s_logits(i, xt):
            lg = psL.tile([P, C], f32, name="lg", tag="lg")
            for q in range(4):
                nc.tensor.matmul(
                    lg[:],
                    xt[:, 2 * q:2 * q + 2, :],
                    Wq[:, 2 * q:2 * q + 2, :],
                    start=(q == 0),
                    stop=False,
                    perf_mode=DR,
                )
            nc.tensor.matmul(lg[:], ones1[:], b_sb[:], start=False, stop=True)
            return lg

        def s_softmax(i, lg, pq):
            # |logits| <= ~10 so exp is safe without max-subtraction
            p_sb = ppool.tile([P, C], f32, name="p_sb", tag="p")
            ssum = spool.tile([P, 1], f32, name="ssum", tag="s")
            nc.scalar.activation(p_sb[:], lg[:], AF.Exp, accum_out=ssum[:])
            rinv = spool.tile([P, 1], f32, name="rinv", tag="r")
            nc.vector.reciprocal(rinv[:], ssum[:])
            nc.vector.tensor_scalar_mul(pq[:, i % 2, :], p_sb[:], rinv[:])

        def s_ptx(pair, pq, xq):
            # PtX += P_pair.T @ X_pair, fp8 DoubleRow over K=256 rows
            for cc in range(2):
                for h in range(2):
                    nc.tensor.matmul(
                        ptx_ps[2 * cc + h][:],
                        pq[:, :, cc * P:(cc + 1) * P],
                        xq[:, :, h * DH:(h + 1) * DH],
                        start=(pair == 0),
                        stop=(pair == NPAIR - 1),
                        perf_mode=DR,
                    )
            # P.T for phase B: 4 transposes (2 tiles x 2 c-chunks), ACT
            # drains fp8 PSUM -> resident bf16 Pt.
            ptp = psP.tile([P, 2, C, 2], fp8, name="ptp", tag="ptp")
            for j in range(2):
                for cc in range(2):
                    nc.tensor.matmul(
                        ptp[:, j, cc * P:(cc + 1) * P, 0],
                        pq[:, j, cc * P:(cc + 1) * P],
                        ident[:],
                        is_transpose=True,
                        start=(j == 0 and cc == 0),
                        stop=(j == 1 and cc == 1),
                    )
            dst = Pt[:, :, 2 * pair * P:(2 * pair + 2) * P].rearrange(
                "p cc (j r) -> p cc j r", j=2
            )
            src = ptp[:, :, :, 0].rearrange("p j (cc r) -> p cc j r", cc=2)
            nc.scalar.copy(dst, src)

        with tc.tile_pool(name="psA", bufs=2, space="PSUM") as psA, \
             tc.tile_pool(name="psP", bufs=1, space="PSUM") as psP, \
             tc.tile_pool(name="psL", bufs=1, space="PSUM") as psL, \
             tc.tile_pool(name="psX", bufs=1, space="PSUM") as psX:
            ptx_ps = [
                psX.tile([P, DH], f32, name=f"ptx_{c}_{h}", tag=f"ptx_{c}_{h}")
                for c in range(2)
                for h in range(2)
            ]
            # Pipeline: at step i run logits(i), transpose(i+1), load(i+2),
            # and ptx for the pair ending at tile i-2. The 2-step skew lets
            # the ACT exp latency hide under PE work.
            state = {}

            def stage_front(i):
                # load + cast for tile i
                if i % 2 == 0:
                    xq = s_load(i)
                else:
                    s_load(i)
                    xq = state[i - 1][0]
                s_cast(i, xq)
                return xq

            xq0 = stage_front(0)
            state[0] = (xq0, s_transpose(0, xq0), None)
            xq1 = stage_front(1)
            state[1] = (xq1, None, None)
            for i in range(NT):
                xq_i, xt_i, _ = state[i]
                lg = s_logits(i, xt_i)
                if i % 2 == 0:
                    pq = pqp.tile([P, 2, C], fp8, name="pq", tag="pq")
                else:
                    pq = state[i - 1][2]
                s_softmax(i, lg, pq)
                state[i] = (xq_i, xt_i, pq)
                if i + 1 < NT:
                    xq_n, _, _ = state[i + 1]
                    state[i + 1] = (xq_n, s_transpose(i + 1, xq_n), None)
                if i + 2 < NT:
                    state[i + 2] = (stage_front(i + 2), None, None)
                if i >= 3 and (i - 2) % 2 == 1:
                    pair = (i - 3) // 2
                    xq_p = state.pop(2 * pair)[0]
                    pq_p = state.pop(2 * pair + 1)[2]
                    s_ptx(pair, pq_p, xq_p)
            # in-loop ptx covers pairs 0..NPAIR-2; finish the last pair
            for pair in (NPAIR - 1,):
                xq_p = state.pop(2 * pair)[0]
                pq_p = state.pop(2 * pair + 1)[2]
                s_ptx(pair, pq_p, xq_p)

            # PSUM -> SBUF -> DRAM bounce, one per D-half, cast to bf16
            # (the PtX rounding is gamma-scaled: invisible in the output).
            for h in range(2):
                s = const.tile([P, 2, DH], bf16, name=f"stg{h}", tag=f"stg{h}")
                nc.vector.tensor_copy(s[:, 0, :], ptx_ps[h][:])
                nc.scalar.copy(s[:, 1, :], ptx_ps[2 + h][:])
                nc.sync.dma_start(
                    ar_in[h].rearrange("(c p) d -> p c d", p=P), s[:]
                )

        # ---- phase B, interleaved with the collectives: AllReduce h=1 is
        # emitted AFTER phase B h=0 so h=0's consumers only wait on the
        # first collective's completion tick, and the second collective
        # runs concurrently with h=0 compute. gamma folded into PtX so the
        # residual is one add. ----
        def ar(h):
            nc.gpsimd.collective_compute(
                "AllReduce",
                mybir.AluOpType.add,
                replica_groups=[list(range(NCORES))],
                ins=[ar_in[h][:].opt()],
                outs=[ar_out[h][:].opt()],
            )

        def phase_b(h, psB, cpool):
            pall = const.tile([P, 2, DH], bf16, name=f"pall{h}", tag=f"stg{h}")
            nc.sync.dma_start(
                pall[:], ar_out[h].rearrange("(c p) d -> p c d", p=P)
            )
            ptxb = const.tile([P, 2, DH], bf16, name=f"ptxb{h}")
            nc.vector.tensor_scalar_mul(ptxb[:], pall[:], GAMMA)
            for i in range(NT):
                cor = psB.tile([P, DH], f32, name="cor", tag="cor")
                for c in range(2):
                    nc.tensor.matmul(
                        cor[:],
                        Pt[:, c, i * P:(i + 1) * P],
                        ptxb[:, c, :],
                        start=(c == 0),
                        stop=(c == 1),
                    )
                o_sb = opool.tile([P, DH], f32, name="o_sb", tag="o")
                if i % 2 == 0:
                    # ACT drains PSUM, DVE adds SBUF+SBUF
                    cors = cpool.tile([P, DH], f32, name="cors", tag="cs")
                    nc.scalar.copy(cors[:], cor[:])
                    nc.vector.tensor_add(
                        o_sb[:], cors[:], Xall[:, i, h * DH:(h + 1) * DH]
                    )
                else:
                    # DVE adds straight from PSUM (one fewer hop)
                    nc.vector.tensor_add(
                        o_sb[:], cor[:], Xall[:, i, h * DH:(h + 1) * DH]
                    )
                nc.sync.dma_start(out[i * P:(i + 1) * P, h * DH:(h + 1) * DH], o_sb[:])

        with tc.tile_pool(name="psB", bufs=8, space="PSUM") as psB, \
             tc.tile_pool(name="cpool", bufs=4) as cpool:
            ar(0)
            phase_b(0, psB, cpool)
            ar(1)
            phase_b(1, psB, cpool)

    nc.finalize()
    return nc


def _run(inputs, trace=False, **kwargs):
    from concourse import bass_utils

    if "nc" not in _cache:
        _cache["nc"] = _build_nc()
    nc = _cache["nc"]

    X = np.ascontiguousarray(np.asarray(inputs["X"], dtype=np.float32))
    W = np.ascontiguousarray(np.asarray(inputs["W"], dtype=np.float32))
    b = np.ascontiguousarray(np.asarray(inputs["b"], dtype=np.float32))
    Wt = np.ascontiguousarray(W.T)

    in_maps = [
        {"X": X[i * NLOC:(i + 1) * NLOC], "Wt": Wt, "b": b} for i in range(NCORES)
    ]
    res = bass_utils.run_bass_kernel_spmd(
        nc, in_maps, core_ids=list(range(NCORES)), trace=trace, **kwargs
    )
    outp = np.concatenate([res.results[i]["out"] for i in range(NCORES)], axis=0)
    return outp, res


def kernel(**inputs):
    outp, _ = _run(inputs, trace=False)
    return outp
